# revision 21
# baseline (speedup 1.0000x reference)
"""BinaryDense kernel for Trainium2: out = sign(x) @ sign(w).

Full shapes: x [8192, 4096] f32, w [4096, 4096] f32 -> out [8192, 4096] f32.

Sharding (8 cores, 2x2x2 grid): x split into 2 row blocks of 4096, w into
2 column blocks of 2048, contraction into 2 halves of 2048.  Each core
computes a [4096, 2048] k-half partial; the host sums partial pairs
(exact: partials are integers |v| <= 2048 held losslessly in f16).

On-device per core (m-tiles of 128 rows, k-pairs of 256 contraction):
  - sign(w) [16 k-tiles] and sign(x) per m-tile via ScalarE Sign cast to
    fp8e4; TensorE matmul in fp8 DoubleRow mode (exact for +-1/0), f32
    PSUM accumulation, f16 eviction; out DMA on the Pool/SWDGE queue so
    stores never head-of-line-block input loads.
  - Rooflines (HW-measured): PE 1024 DR matmuls x ~213ns = ~218us; DMA
    64 MiB / 337 GB/s = ~199us.  A cold start pays an unavoidable ~25us
    ramp while w streams, so the benchmark loop runs with
    staggered_reset=True (no all-engine barrier at the back edge),
    double-buffered sw / leading sx tiles, and the body unrolled 4x per
    trip, letting iteration i+1's w/x streams run under iteration i's
    matmul tail.  Steady-state period ~= max(PE, DMA) + sync slack.
  - Measured loop-slope: 311us (prior 4x2x1 session) -> 275us
    (staggered+2x2x2) -> 255us (unroll=2) -> 239-245us (unroll=4).
    Benches pin PYTHONHASHSEED=0: the Tile scheduler's heap
    tie-breaking is hash-order dependent (+-30us schedule variance).
  - A post-schedule IR pass drops back-to-back-identical PE Ldweights
    (the nb-inner loop reuses each stationary 4x).
"""

import sys

if "/opt/trn_rl_repo" not in sys.path:
    sys.path.insert(0, "/opt/trn_rl_repo")

import numpy as np

P = 128
M_FULL, K_FULL, N_FULL = 8192, 4096, 4096
GRID_I, GRID_J, GRID_L = 2, 2, 2
M_SH = M_FULL // GRID_I  # 4096
N_SH = N_FULL // GRID_J  # 2048
K_SH = K_FULL // GRID_L  # 2048
NBANK = 512  # psum bank free dim (f32)

_PROGRAM_CACHE: dict = {}


def calibrate_cost_model():
    """Patch TRN2Spec so the tile scheduler plans with HW-measured costs.

    Measured on these cores 2026-08-08: DR fp8 matmul N=512 ~213ns (the
    stock model says 107); DMA 337 GB/s effective (stock 360).  The
    patch only affects scheduling heuristics, not semantics.
    """
    from concourse.hw_specs import TRN2Spec

    TRN2Spec.PE_CYCLE = 1e9 / 1.2e9
    TRN2Spec.PE_CYCLE_PSTATE_MID = 1e9 / 1.2e9
    TRN2Spec.PE_CYCLE_PSTATE_LOW = 1e9 / 0.9e9
    TRN2Spec.DMA_BUS_BYTES_PER_NS_PER_ENGINE = 337e9 / 16 / 1e9


def build_program(
    k_full=K_SH,
    m_sh=M_SH,
    n_sh=N_SH,
    mm_dtype_name="float8e4",
    out_dtype_name="float16",
    loop_n=0,
    unroll=4,  # kernel-body copies per For_i trip (auto-shrunk to divide
               # loop_n); amortizes the ~46us trip-edge sync cost
    staggered=True,  # barrier-free loop back edge (cross-iter pipelining)
    stage_mts=None,  # explicit staggered-reset stage boundaries (None =
                     # Tile's equal-instruction-count split, measured best)
    x_lead=3,  # leading x m-tiles in pinned double-buffered slots
    x_ilv=4,  # stage one leading x per this many w k-tiles
    sw_bufs=2,  # double-buffer sw so next iter's w stream overlaps tail
    sxpro_bufs=2,
    wstage_bufs=3,
    xstage_bufs=3,
    sxt_bufs=3,
    out_bufs=3,
):
    import contextlib

    import concourse.mybir as mybir
    import concourse.tile as tile
    from concourse import bacc

    calibrate_cost_model()

    f32 = mybir.dt.float32
    mmdt = getattr(mybir.dt, mm_dtype_name)
    odt = getattr(mybir.dt, out_dtype_name)

    kt_n = k_full // P  # 16 k-tiles
    kp_n = kt_n // 2  # 8 k-pairs
    mt_n = m_sh // P  # 32 m-tiles
    nb_n = n_sh // NBANK  # 4 psum banks per m-tile

    trips = loop_n
    if loop_n:
        while loop_n % unroll:  # fall back to a divisor of loop_n
            unroll -= 1
        trips = loop_n // unroll

    nc = bacc.Bacc(
        "TRN2", target_bir_lowering=False, debug=False, num_devices=8
    )

    xt = nc.dram_tensor(
        "xt", [mt_n, P, kt_n * P], f32, kind="ExternalInput"
    ).ap()
    w = nc.dram_tensor("w", [k_full, n_sh], f32, kind="ExternalInput").ap()
    out = nc.dram_tensor("out", [m_sh, n_sh], odt, kind="ExternalOutput").ap()
    w_t = w.rearrange("(ko p) n -> p ko n", p=P)
    out_t = out.rearrange("(mo p) n -> p mo n", p=P)

    with tile.TileContext(nc) as tc:
        with (
            tc.tile_pool(name="swpool", bufs=sw_bufs) as swpool,
            tc.tile_pool(name="wstage", bufs=wstage_bufs) as wstage,
            tc.tile_pool(name="xstage", bufs=xstage_bufs) as xstage,
            tc.tile_pool(name="sxpro", bufs=sxpro_bufs) as sxpro,
            tc.tile_pool(name="sxt", bufs=sxt_bufs) as sxtpool,
            tc.tile_pool(name="outpool", bufs=out_bufs) as outpool,
            tc.tile_pool(name="psum", bufs=8, space="PSUM") as psump,
            tc.For_i(0, trips, 1, staggered_reset=staggered)
            if loop_n
            else contextlib.nullcontext(),
        ):
            def stage_x(u, mt, pool, tag):
                xst = xstage.tile([P, kt_n * P], f32, tag="xst")
                nc.sync.dma_start(xst, xt[mt, :, :])
                sx = pool.tile(
                    [P, kt_n, P], mmdt, tag=tag, name=f"sx_{u}_{mt}"
                )
                nc.scalar.sign(
                    sx, xst.rearrange("p (ko m) -> p ko m", m=P)
                )
                return sx

            def emit_body(u):
                sw = swpool.tile(
                    [P, kt_n, n_sh], mmdt, tag="sw", name=f"sw_{u}"
                )

                def stage_w(kt):
                    wst = wstage.tile([P, n_sh], f32, tag="wst")
                    nc.sync.dma_start(wst, w_t[:, kt, :])
                    nc.scalar.sign(sw[:, kt, :], wst)

                # input staging emission: leading x tiles interleave into
                # the w stream (SP issues in priority=emission order)
                sx_map = {}
                nxt = 0
                sx_map[nxt] = stage_x(u, nxt, sxpro, f"sxp{nxt}")
                nxt += 1
                for kt in range(kt_n):
                    stage_w(kt)
                    if (kt + 1) % x_ilv == 0 and nxt < x_lead:
                        sx_map[nxt] = stage_x(u, nxt, sxpro, f"sxp{nxt}")
                        nxt += 1
                while nxt < x_lead:
                    sx_map[nxt] = stage_x(u, nxt, sxpro, f"sxp{nxt}")
                    nxt += 1

                for mt in range(mt_n):
                    if loop_n and staggered and mt in (stage_mts or ()):
                        tc.stage_boundary()
                    sx = (
                        sx_map[mt]
                        if mt in sx_map
                        else stage_x(u, mt, sxtpool, "sxt")
                    )
                    ps = [
                        psump.tile(
                            [P, NBANK], f32, tag="ps", name=f"ps{u}_{mt}n{nb}"
                        )
                        for nb in range(nb_n)
                    ]
                    for kp in range(kp_n):
                        for nb in range(nb_n):
                            nc.tensor.matmul(
                                ps[nb],
                                lhsT=sx[:, 2 * kp : 2 * kp + 2, :],
                                rhs=sw[
                                    :,
                                    2 * kp : 2 * kp + 2,
                                    nb * NBANK : (nb + 1) * NBANK,
                                ],
                                start=(kp == 0),
                                stop=(kp == kp_n - 1),
                                perf_mode=mybir.MatmulPerfMode.DoubleRow,
                            )
                    outt = outpool.tile([P, n_sh], odt, tag="outt")
                    for nb in range(nb_n):
                        nc.vector.tensor_copy(
                            outt[:, nb * NBANK : (nb + 1) * NBANK], ps[nb]
                        )
                    nc.gpsimd.dma_start(out_t[:, mt, :], outt)

            for u in range(unroll if loop_n else 1):
                emit_body(u)

    _dedup_ldweights(nc)
    nc.compile()
    return nc


def _dedup_ldweights(nc):
    """Drop PE Ldweights that reload the exact stationary already resident.

    Tile's lowering emits one Ldweights per matmul; with an nb-inner loop
    the same lhsT is reloaded 4x back-to-back.  Only instructions with
    empty sync_info are dropped, and any other PE instruction invalidates
    the tracked stationary, so semaphore semantics are preserved.
    """
    removed = 0
    for blk in nc.m.functions[0].blocks:
        il = blk.instructions
        last_key = None
        i = 0
        while i < len(il):
            inst = il[i]
            t = type(inst).__name__
            if t == "InstLdweights":
                key = (
                    str(inst.ins[0]),
                    str(inst.perf_mode),
                    str(inst.is_transpose),
                    str(inst.tile_position),
                    str(inst.tile_size),
                )
                si = inst.sync_info
                empty = si is None or (
                    not list(si.on_wait) and not list(si.on_update)
                )
                if key == last_key and empty:
                    il.pop(i)
                    removed += 1
                    continue
                last_key = key
            elif t == "InstMatmult":
                pass
            elif str(getattr(inst, "engine", "")) == "EngineType.PE":
                last_key = None
            i += 1
    return removed


def _get_program():
    key = "main"
    if key not in _PROGRAM_CACHE:
        _PROGRAM_CACHE[key] = build_program()
    return _PROGRAM_CACHE[key]


def pack_xt(x_block: np.ndarray) -> np.ndarray:
    """[m_sh, k] row block -> [mt, p, ko*P + m] with p = k % P (partition)."""
    m_sh, k_full = x_block.shape
    v = x_block.reshape(m_sh // P, P, k_full // P, P)  # [mt, m, ko, p]
    v = v.transpose(0, 3, 2, 1)  # [mt, p, ko, m]
    return np.ascontiguousarray(v).reshape(m_sh // P, P, k_full)


def make_in_maps(x: np.ndarray, w: np.ndarray):
    """Shard full inputs into per-core in_maps (2 m x 2 n x 2 k grid).

    Core c = i*4 + j*2 + l gets x rows [i], k-half [l] and w k-half [l],
    n cols [j]; it computes the k-half partial of out block (i, j).
    """
    x = np.asarray(x, dtype=np.float32)
    w = np.asarray(w, dtype=np.float32)
    xt_shards = {}
    for i in range(GRID_I):
        for l in range(GRID_L):
            xt_shards[(i, l)] = pack_xt(
                x[i * M_SH : (i + 1) * M_SH, l * K_SH : (l + 1) * K_SH]
            )
    w_shards = {}
    for j in range(GRID_J):
        for l in range(GRID_L):
            w_shards[(j, l)] = np.ascontiguousarray(
                w[l * K_SH : (l + 1) * K_SH, j * N_SH : (j + 1) * N_SH]
            )
    in_maps = []
    for c in range(8):
        i, rest = divmod(c, 4)
        j, l = divmod(rest, 2)
        in_maps.append({"xt": xt_shards[(i, l)], "w": w_shards[(j, l)]})
    return in_maps


def assemble(results):
    """Sum k-half partials and place blocks into the full [8192, 4096] out."""
    out = np.empty((M_FULL, N_FULL), dtype=np.float32)
    for i in range(GRID_I):
        for j in range(GRID_J):
            c0 = i * 4 + j * 2
            blk = results[c0]["out"].astype(np.float32) + results[c0 + 1][
                "out"
            ].astype(np.float32)
            out[i * M_SH : (i + 1) * M_SH, j * N_SH : (j + 1) * N_SH] = blk
    return out


def run_on_device(x, w, trace=False, **kwargs):
    from concourse.bass_utils import run_bass_kernel_spmd

    nc = _get_program()
    in_maps = make_in_maps(x, w)
    res = run_bass_kernel_spmd(
        nc, in_maps, core_ids=list(range(8)), trace=trace, **kwargs
    )
    return res


def kernel(x: np.ndarray, w: np.ndarray) -> np.ndarray:
    res = run_on_device(x, w)
    return assemble(res.results)


# revision 23
# speedup vs baseline: 1.0362x; 1.0362x over previous
"""BinaryDense kernel for Trainium2: out = sign(x) @ sign(w).

Full shapes: x [8192, 4096] f32, w [4096, 4096] f32 -> out [8192, 4096] f32.

Sharding (8 cores, 2x2x2 grid): x split into 2 row blocks of 4096, w into
2 column blocks of 2048, contraction into 2 halves of 2048.  Each core
computes a [4096, 2048] k-half partial; the host sums partial pairs
(exact: partials are integers |v| <= 2048 held losslessly in f16).

On-device per core (m-tiles of 128 rows, k-pairs of 256 contraction):
  - sign(w) [16 k-tiles] and sign(x) per m-tile via ScalarE Sign cast to
    fp8e4; TensorE matmul in fp8 DoubleRow mode (exact for +-1/0), f32
    PSUM accumulation, f16 eviction; out DMA on the Pool/SWDGE queue so
    stores never head-of-line-block input loads.
  - Rooflines (HW-measured): PE 1024 DR matmuls x ~213ns = ~218us; DMA
    64 MiB / 337 GB/s = ~199us.  A cold start pays an unavoidable ~25us
    ramp while w streams, so the benchmark loop runs with
    staggered_reset=True (no all-engine barrier at the back edge),
    double-buffered sw / leading sx tiles, and the body unrolled 4x per
    trip, letting iteration i+1's w/x streams run under iteration i's
    matmul tail.  Steady-state period ~= max(PE, DMA) + sync slack.
  - Measured loop-slope: 311us (prior 4x2x1 session) -> 275us
    (staggered+2x2x2) -> 255us (unroll=2) -> 239-245us (unroll=4).
    Benches pin PYTHONHASHSEED=0: the Tile scheduler's heap
    tie-breaking is hash-order dependent (+-30us schedule variance).
  - A post-schedule IR pass drops back-to-back-identical PE Ldweights
    (the nb-inner loop reuses each stationary 4x).
"""

import sys

if "/opt/trn_rl_repo" not in sys.path:
    sys.path.insert(0, "/opt/trn_rl_repo")

import numpy as np

P = 128
M_FULL, K_FULL, N_FULL = 8192, 4096, 4096
GRID_I, GRID_J, GRID_L = 2, 2, 2
M_SH = M_FULL // GRID_I  # 4096
N_SH = N_FULL // GRID_J  # 2048
K_SH = K_FULL // GRID_L  # 2048
NBANK = 512  # psum bank free dim (f32)

_PROGRAM_CACHE: dict = {}


def calibrate_cost_model():
    """Patch TRN2Spec so the tile scheduler plans with HW-measured costs.

    Measured on these cores 2026-08-08: DR fp8 matmul N=512 ~213ns (the
    stock model says 107); DMA 337 GB/s effective (stock 360).  The
    patch only affects scheduling heuristics, not semantics.
    """
    from concourse.hw_specs import TRN2Spec

    TRN2Spec.PE_CYCLE = 1e9 / 1.2e9
    TRN2Spec.PE_CYCLE_PSTATE_MID = 1e9 / 1.2e9
    TRN2Spec.PE_CYCLE_PSTATE_LOW = 1e9 / 0.9e9
    TRN2Spec.DMA_BUS_BYTES_PER_NS_PER_ENGINE = 337e9 / 16 / 1e9


def build_program(
    k_full=K_SH,
    m_sh=M_SH,
    n_sh=N_SH,
    mm_dtype_name="float8e4",
    out_dtype_name="float16",
    loop_n=0,
    unroll=4,  # kernel-body copies per For_i trip (auto-shrunk to divide
               # loop_n); amortizes the ~46us trip-edge sync cost
    staggered=True,  # barrier-free loop back edge (cross-iter pipelining)
    loop_hints=False,  # back-edge branch-prefetch hints on all engines
    stage_mts=None,  # explicit staggered-reset stage boundaries (None =
                     # Tile's equal-instruction-count split, measured best)
    x_lead=3,  # leading x m-tiles in pinned double-buffered slots
    x_ilv=4,  # stage one leading x per this many w k-tiles
    sw_bufs=2,  # double-buffer sw so next iter's w stream overlaps tail
    sxpro_bufs=2,
    wstage_bufs=3,
    xstage_bufs=3,
    sxt_bufs=3,
    out_bufs=3,
):
    import contextlib

    import concourse.mybir as mybir
    import concourse.tile as tile
    from concourse import bacc

    calibrate_cost_model()

    f32 = mybir.dt.float32
    mmdt = getattr(mybir.dt, mm_dtype_name)
    odt = getattr(mybir.dt, out_dtype_name)

    kt_n = k_full // P  # 16 k-tiles
    kp_n = kt_n // 2  # 8 k-pairs
    mt_n = m_sh // P  # 32 m-tiles
    nb_n = n_sh // NBANK  # 4 psum banks per m-tile

    trips = loop_n
    if loop_n:
        while loop_n % unroll:  # fall back to a divisor of loop_n
            unroll -= 1
        trips = loop_n // unroll

    nc = bacc.Bacc(
        "TRN2", target_bir_lowering=False, debug=False, num_devices=8
    )

    xt = nc.dram_tensor(
        "xt", [mt_n, P, kt_n * P], f32, kind="ExternalInput"
    ).ap()
    w = nc.dram_tensor("w", [k_full, n_sh], f32, kind="ExternalInput").ap()
    out = nc.dram_tensor("out", [m_sh, n_sh], odt, kind="ExternalOutput").ap()
    w_t = w.rearrange("(ko p) n -> p ko n", p=P)
    out_t = out.rearrange("(mo p) n -> p mo n", p=P)

    with tile.TileContext(nc) as tc:
        with (
            tc.tile_pool(name="swpool", bufs=sw_bufs) as swpool,
            tc.tile_pool(name="wstage", bufs=wstage_bufs) as wstage,
            tc.tile_pool(name="xstage", bufs=xstage_bufs) as xstage,
            tc.tile_pool(name="sxpro", bufs=sxpro_bufs) as sxpro,
            tc.tile_pool(name="sxt", bufs=sxt_bufs) as sxtpool,
            tc.tile_pool(name="outpool", bufs=out_bufs) as outpool,
            tc.tile_pool(name="psum", bufs=8, space="PSUM") as psump,
            tc.For_i(
                0,
                trips,
                1,
                staggered_reset=staggered,
                hint_engines=list(mybir.ALL_ENGINES) if loop_hints else (),
            )
            if loop_n
            else contextlib.nullcontext(),
        ):
            def stage_x(u, mt, pool, tag):
                xst = xstage.tile([P, kt_n * P], f32, tag="xst")
                nc.sync.dma_start(xst, xt[mt, :, :])
                sx = pool.tile(
                    [P, kt_n, P], mmdt, tag=tag, name=f"sx_{u}_{mt}"
                )
                nc.scalar.sign(
                    sx, xst.rearrange("p (ko m) -> p ko m", m=P)
                )
                return sx

            def emit_body(u):
                sw = swpool.tile(
                    [P, kt_n, n_sh], mmdt, tag="sw", name=f"sw_{u}"
                )

                def stage_w(kt):
                    wst = wstage.tile([P, n_sh], f32, tag="wst")
                    nc.sync.dma_start(wst, w_t[:, kt, :])
                    nc.scalar.sign(sw[:, kt, :], wst)

                # input staging emission: leading x tiles interleave into
                # the w stream (SP issues in priority=emission order)
                sx_map = {}
                nxt = 0
                sx_map[nxt] = stage_x(u, nxt, sxpro, f"sxp{nxt}")
                nxt += 1
                for kt in range(kt_n):
                    stage_w(kt)
                    if (kt + 1) % x_ilv == 0 and nxt < x_lead:
                        sx_map[nxt] = stage_x(u, nxt, sxpro, f"sxp{nxt}")
                        nxt += 1
                while nxt < x_lead:
                    sx_map[nxt] = stage_x(u, nxt, sxpro, f"sxp{nxt}")
                    nxt += 1

                for mt in range(mt_n):
                    if loop_n and staggered and mt in (stage_mts or ()):
                        tc.stage_boundary()
                    sx = (
                        sx_map[mt]
                        if mt in sx_map
                        else stage_x(u, mt, sxtpool, "sxt")
                    )
                    ps = [
                        psump.tile(
                            [P, NBANK], f32, tag="ps", name=f"ps{u}_{mt}n{nb}"
                        )
                        for nb in range(nb_n)
                    ]
                    for kp in range(kp_n):
                        for nb in range(nb_n):
                            nc.tensor.matmul(
                                ps[nb],
                                lhsT=sx[:, 2 * kp : 2 * kp + 2, :],
                                rhs=sw[
                                    :,
                                    2 * kp : 2 * kp + 2,
                                    nb * NBANK : (nb + 1) * NBANK,
                                ],
                                start=(kp == 0),
                                stop=(kp == kp_n - 1),
                                perf_mode=mybir.MatmulPerfMode.DoubleRow,
                            )
                    outt = outpool.tile([P, n_sh], odt, tag="outt")
                    for nb in range(nb_n):
                        nc.vector.tensor_copy(
                            outt[:, nb * NBANK : (nb + 1) * NBANK], ps[nb]
                        )
                    nc.gpsimd.dma_start(out_t[:, mt, :], outt)

            for u in range(unroll if loop_n else 1):
                emit_body(u)

    _dedup_ldweights(nc)
    nc.compile()
    return nc


def _dedup_ldweights(nc):
    """Drop PE Ldweights that reload the exact stationary already resident.

    Tile's lowering emits one Ldweights per matmul; with an nb-inner loop
    the same lhsT is reloaded 4x back-to-back.  Only instructions with
    empty sync_info are dropped, and any other PE instruction invalidates
    the tracked stationary, so semaphore semantics are preserved.
    """
    removed = 0
    for blk in nc.m.functions[0].blocks:
        il = blk.instructions
        last_key = None
        i = 0
        while i < len(il):
            inst = il[i]
            t = type(inst).__name__
            if t == "InstLdweights":
                key = (
                    str(inst.ins[0]),
                    str(inst.perf_mode),
                    str(inst.is_transpose),
                    str(inst.tile_position),
                    str(inst.tile_size),
                )
                si = inst.sync_info
                empty = si is None or (
                    not list(si.on_wait) and not list(si.on_update)
                )
                if key == last_key and empty:
                    il.pop(i)
                    removed += 1
                    continue
                last_key = key
            elif t == "InstMatmult":
                pass
            elif str(getattr(inst, "engine", "")) == "EngineType.PE":
                last_key = None
            i += 1
    return removed


def _get_program():
    key = "main"
    if key not in _PROGRAM_CACHE:
        _PROGRAM_CACHE[key] = build_program()
    return _PROGRAM_CACHE[key]


def pack_xt(x_block: np.ndarray) -> np.ndarray:
    """[m_sh, k] row block -> [mt, p, ko*P + m] with p = k % P (partition)."""
    m_sh, k_full = x_block.shape
    v = x_block.reshape(m_sh // P, P, k_full // P, P)  # [mt, m, ko, p]
    v = v.transpose(0, 3, 2, 1)  # [mt, p, ko, m]
    return np.ascontiguousarray(v).reshape(m_sh // P, P, k_full)


def make_in_maps(x: np.ndarray, w: np.ndarray):
    """Shard full inputs into per-core in_maps (2 m x 2 n x 2 k grid).

    Core c = i*4 + j*2 + l gets x rows [i], k-half [l] and w k-half [l],
    n cols [j]; it computes the k-half partial of out block (i, j).
    """
    x = np.asarray(x, dtype=np.float32)
    w = np.asarray(w, dtype=np.float32)
    xt_shards = {}
    for i in range(GRID_I):
        for l in range(GRID_L):
            xt_shards[(i, l)] = pack_xt(
                x[i * M_SH : (i + 1) * M_SH, l * K_SH : (l + 1) * K_SH]
            )
    w_shards = {}
    for j in range(GRID_J):
        for l in range(GRID_L):
            w_shards[(j, l)] = np.ascontiguousarray(
                w[l * K_SH : (l + 1) * K_SH, j * N_SH : (j + 1) * N_SH]
            )
    in_maps = []
    for c in range(8):
        i, rest = divmod(c, 4)
        j, l = divmod(rest, 2)
        in_maps.append({"xt": xt_shards[(i, l)], "w": w_shards[(j, l)]})
    return in_maps


def assemble(results):
    """Sum k-half partials and place blocks into the full [8192, 4096] out."""
    out = np.empty((M_FULL, N_FULL), dtype=np.float32)
    for i in range(GRID_I):
        for j in range(GRID_J):
            c0 = i * 4 + j * 2
            blk = results[c0]["out"].astype(np.float32) + results[c0 + 1][
                "out"
            ].astype(np.float32)
            out[i * M_SH : (i + 1) * M_SH, j * N_SH : (j + 1) * N_SH] = blk
    return out


def run_on_device(x, w, trace=False, **kwargs):
    from concourse.bass_utils import run_bass_kernel_spmd

    nc = _get_program()
    in_maps = make_in_maps(x, w)
    res = run_bass_kernel_spmd(
        nc, in_maps, core_ids=list(range(8)), trace=trace, **kwargs
    )
    return res


def kernel(x: np.ndarray, w: np.ndarray) -> np.ndarray:
    res = run_on_device(x, w)
    return assemble(res.results)
